# revision 29
# baseline (speedup 1.0000x reference)
"""MoE experts kernel for Trainium2 (Bass/Tile), expert-parallel across 8 NeuronCores.

Problem: nn_CompressedMoeExperts — T=2048 tokens, D=1024, FF=1536, E=8 experts,
top-k=2.  out[t] = sum_e combine[e,t] * (silu(h[t] @ Wg[e].T) * (h[t] @ Wu[e].T)) @ Wd[e].T

Sharding: expert-parallel with FF-split load balancing.  Each expert's MLP is
split into two independent shards along the FF dimension (rows of Wg/Wu,
columns of Wd — their partial down-projection outputs simply add).  The 16
shards are sorted by routed-token count and dealt out so every core gets one
"big" and one "small" shard.  Token dispatch (gather by top_k_index) and the
weighted combine scatter-add happen on the host as part of sharding/
unsharding; the combine weight itself is applied on-device.

Matmul operands are fp16 (halves HBM traffic vs fp32, 1 cycle/row on the PE,
fast weight loads), accumulating in fp32 PSUM.  Values are far inside fp16
range and the 10-bit mantissa keeps L2 relative error ~5e-4.

v2 optimizations over the 91.3us baseline (trace-driven):
- Token capacities padded to 32 (not 128): Cs=(544,512) instead of (640,512)
  for the max routed count of 528 → 9K fewer PE cycles (~3.8us).
- Each dma_start costs ~620ns of DIRECT2D issue time serialized on its
  issuing engine's sequencer (72 issues = 44us on Sync in the baseline, and
  ~8us of issue latency before the first weight byte moved).  Fixes: gate+up
  merged into one wgu feed (12 issues not 24), y outputs merged across the
  two D-halves and written fp16 (9-10 issues not 18-20), and issue load split
  across the two HWDGE engines: sync carries wgu/xt/wt, scalar carries wd + y.
- Phase-2 eviction moved from Vector (tensor_scalar_mul, 751ns/tile PSUM read)
  to the Scalar engine as activation(Copy, scale=wt_column) straight to fp16;
  the y DMA is issued by scalar immediately after (same-queue, no cross-engine
  semaphore), halving output bytes.
- Warmup trimmed to 8 matmuls (3.4us at the HAM-gated 1.2GHz exactly covers
  the clock ramp); xt1/wt1 issue hoisted into the startup batch so the s1
  token feed can never serialize behind phase-2 y traffic.
"""

import os
import sys

sys.path.insert(0, "/opt/trn_rl_repo")

import numpy as np

import concourse.bass as bass
import concourse.mybir as mybir
import concourse.tile as tile
from concourse import bacc
from concourse.bass_utils import run_bass_kernel_spmd

# Fixed problem shape
T, D, FF, E, TOPK = 2048, 1024, 1536, 8, 2
P = 128
DSUB = D // P     # 8   k-subtiles over the D contraction
FBLK = FF // P    # 12  128-row blocks over the full FF dimension
NSPLIT = 2
FBH = FBLK // NSPLIT   # 128-row FF blocks per shard
FH = FF // NSPLIT      # FF columns per shard
NDN = 512         # free-dim tile for the down projection
NDT = D // NDN    # 2

F32 = mybir.dt.float32
F16 = mybir.dt.float16

_program_cache: dict[tuple, "bass.Bass"] = {}
last_results = None  # BassKernelResults of the most recent run (for profiling)


def _chunks(C: int) -> list[int]:
    """Split C (multiple of 32) into matmul moving-dim chunks of <=512
    (PSUM bank limit for fp32 accumulation), sizes multiples of 32."""
    n = -(-C // 512)
    base = C // n
    base -= base % 32
    out = [base] * n
    rem = C - base * n  # multiple of 32
    i = 0
    while rem > 0:
        add = min(32, rem)
        out[i % n] += add
        rem -= add
        i += 1
    return sorted(out)  # smallest first: quickest start on freshly-DMAed data


def _build_program(Cs: tuple) -> "bass.Bass":
    nc = bacc.Bacc(None, target_bir_lowering=False)

    ntbs = [-(-C // P) for C in Cs]

    xt_d = [
        nc.dram_tensor(f"xt{s}", [P, DSUB, Cs[s]], F16, kind="ExternalInput")
        for s in range(NSPLIT)
    ]
    wgu_d = nc.dram_tensor("wgu", [P, FBLK, 2, DSUB, P], F16, kind="ExternalInput")
    wd_d = nc.dram_tensor("wd", [P, FBLK, NDT, NDN], F16, kind="ExternalInput")
    wt_d = [
        nc.dram_tensor(f"wt{s}", [P, ntbs[s]], F32, kind="ExternalInput")
        for s in range(NSPLIT)
    ]
    y_d = [
        nc.dram_tensor(f"y{s}", [Cs[s], D], F16, kind="ExternalOutput")
        for s in range(NSPLIT)
    ]

    with tile.TileContext(nc) as tc:
        with (
            tc.tile_pool(name="const", bufs=1) as const_pool,
            tc.tile_pool(name="actp", bufs=1) as act_pool,
            tc.tile_pool(name="sgp", bufs=3) as sg_pool,
            tc.tile_pool(name="yp", bufs=3) as y_pool,
            tc.tile_pool(name="psum", bufs=2, space="PSUM") as psum_pool,
            tc.tile_pool(name="psum_y", bufs=3, space="PSUM") as psum_y_pool,
            tc.tile_pool(name="psum_w", bufs=1, space="PSUM") as psum_w_pool,
        ):
            # HAM pre-warm: 8 dummy matmuls (only dep: the memset) cover the
            # 1.2GHz->2.4GHz clock ramp (~3.4us of PE activity) while the
            # first DMAs stage.
            warm_in = const_pool.tile([P, NDN], F16)
            nc.vector.memset(warm_in[:], 0.0)
            warm_ps = psum_w_pool.tile([P, NDN], F32)
            for _ in range(8):
                nc.tensor.matmul(warm_ps[:], warm_in[:, :P], warm_in[:])

            # Startup DMA batch.  sync (HWDGE) carries wgu/xt/wt; scalar
            # (also HWDGE) carries half of xt0, wd, and later the y outputs.
            # Each dma_start costs ~620ns of DIRECT2D issue time serialized on
            # its engine, while the transfer itself is chopped into
            # per-partition-row descriptors fanned across all 16 hw queues —
            # so FEW, BIG dma_starts both issue fast and use full bandwidth.
            # The whole gate/up table lives in SBUF (6 MB) and streams in as
            # 2-block (1 MB) pieces paced just ahead of the PE.
            # Fine-grained startup: the first real matmul group waits only on
            # wgu block 0 (512K) + xt0 k=0:2 (370K); later k-slices and blocks
            # stream in behind, half on the scalar engine's parallel queue.
            wgu_sb = const_pool.tile([P, FBLK, 2, DSUB, P], F16, name="wgu_sb")
            nc.sync.dma_start(wgu_sb[:, 0:1], wgu_d[:, 0:1])
            xt = [
                const_pool.tile([P, DSUB, Cs[s]], F16, name=f"xt{s}")
                for s in range(NSPLIT)
            ]
            nc.sync.dma_start(xt[0][:, 0:2], xt_d[0][:, 0:2])
            nc.scalar.dma_start(xt[0][:, 4:6], xt_d[0][:, 4:6])
            nc.sync.dma_start(xt[0][:, 2:4], xt_d[0][:, 2:4])
            nc.scalar.dma_start(xt[0][:, 6:8], xt_d[0][:, 6:8])
            wt_sb = [
                const_pool.tile([P, ntbs[s]], F32, name=f"wt{s}") for s in range(NSPLIT)
            ]
            nc.sync.dma_start(wgu_sb[:, 1:2], wgu_d[:, 1:2])
            nc.sync.dma_start(wgu_sb[:, 2:3], wgu_d[:, 2:3])
            nc.sync.dma_start(wgu_sb[:, 3:4], wgu_d[:, 3:4])
            nc.sync.dma_start(wt_sb[0][:], wt_d[0][:])

            wd_sb = const_pool.tile([P, FBLK, NDT, NDN], F16)

            act = [
                act_pool.tile([P, FBH, Cs[s]], F16, name=f"act{s}")
                for s in range(NSPLIT)
            ]

            def phase1(s, extra=()):
                # `extra`: background DMA issues, one per fb block, threaded
                # through the sync queue behind this slot's weight stream.
                C = Cs[s]
                csizes = _chunks(C)
                extra = list(extra)
                for fbl in range(FBH):
                    fb = s * FBH + fbl
                    if fbl < len(extra) and extra[fbl] is not None:
                        extra[fbl]()

                    # slot 0 consumes k-slices in DMA-arrival order: the
                    # scalar-issued pieces (k=4:8) land before sync's (k=0:4),
                    # so the first real matmul starts the moment warmup ends.
                    k_order = (4, 5, 6, 7, 0, 1, 2, 3) if s == 0 else range(DSUB)
                    col = 0
                    for cs in csizes:
                        pg = psum_pool.tile([P, NDN], F32, tag="pg", name="pg")[:, :cs]
                        pu = psum_pool.tile([P, NDN], F32, tag="pu", name="pu")[:, :cs]
                        for i, k in enumerate(k_order):
                            nc.tensor.matmul(
                                pg,
                                wgu_sb[:, fb, 0, k, :],
                                xt[s][:, k, col : col + cs],
                                start=(i == 0),
                                stop=(i == DSUB - 1),
                            )
                        for i, k in enumerate(k_order):
                            nc.tensor.matmul(
                                pu,
                                wgu_sb[:, fb, 1, k, :],
                                xt[s][:, k, col : col + cs],
                                start=(i == 0),
                                stop=(i == DSUB - 1),
                            )
                        sg = sg_pool.tile([P, NDN], F32, tag="sg", name="sg")[:, :cs]
                        nc.scalar.activation(
                            sg, pg, mybir.ActivationFunctionType.Silu
                        )
                        nc.vector.tensor_mul(act[s][:, fbl, col : col + cs], sg, pu)
                        col += cs

            def phase2(s, extra=()):
                # y[t, d] = (actT.T @ WdT_half) * combine_weight[t], evicted on
                # the scalar engine (Copy with per-partition scale) straight to
                # fp16; y DMA issued from scalar right after (same queue).
                # `extra`: background input DMAs to issue one-per-tb — phase2
                # consumes no input stream, so it is the bandwidth-quiet
                # window to stage the next slot's feeds.
                C = Cs[s]
                extra = list(extra)
                per_tb = -(-len(extra) // max(1, ntbs[s]))
                for tb in range(ntbs[s]):
                    for _ in range(per_tb):
                        if extra:
                            extra.pop(0)()
                    rows = min(P, C - tb * P)
                    y_sb = y_pool.tile([P, D], F16, tag="ysb")
                    for dti in range(NDT):
                        py = psum_y_pool.tile([P, NDN], F32, tag="py")
                        for fs in range(FBH):
                            nc.tensor.matmul(
                                py[:rows],
                                act[s][:, fs, tb * P : tb * P + rows],
                                wd_sb[:, s * FBH + fs, dti, :],
                                start=(fs == 0),
                                stop=(fs == FBH - 1),
                            )
                        nc.scalar.activation(
                            y_sb[:rows, dti * NDN : (dti + 1) * NDN],
                            py[:rows],
                            mybir.ActivationFunctionType.Copy,
                            scale=wt_sb[s][:rows, tb : tb + 1],
                        )
                    # y DMA issued from sync (idle during phase 2) so its
                    # DIRECT2D overlaps the scalar evictions
                    nc.sync.dma_start(
                        y_d[s][tb * P : tb * P + rows, :], y_sb[:rows]
                    )

            def _iss_xt1(j0, j1):
                return lambda: nc.sync.dma_start(
                    xt[1][:, j0:j1], xt_d[1][:, j0:j1]
                )

            def _iss_wt1():
                return lambda: nc.sync.dma_start(wt_sb[1][:], wt_d[1][:])

            def _iss_wgu(b0, b1):
                return lambda: nc.sync.dma_start(
                    wgu_sb[:, b0:b1], wgu_d[:, b0:b1]
                )

            def _iss_wd(b0, b1):
                return lambda: nc.scalar.dma_start(
                    wd_sb[:, b0:b1], wd_d[:, b0:b1]
                )

            # Input-stream pacing (~1 MB per issue, ~2 pieces of lookahead):
            # the startup batch carries wgu 0:4 + xt0; phase1(0) threads in
            # wgu 4:8, the xt1 halves, and (on scalar) wd 0:6 which phase2(0)
            # needs; phase2(0)'s input-quiet window stages wgu 8:10;
            # phase1(1) only needs wgu 10:12 + wd 6:12.
            def _pair(a, b):
                return lambda: (a(), b())

            phase1(0, extra=[_iss_wgu(4, 6), _iss_xt1(0, 4),
                             _pair(_iss_wgu(6, 8), _iss_wd(0, 2)),
                             _pair(_iss_xt1(4, 8), _iss_wd(2, 4)),
                             _iss_wd(4, 6), _iss_wt1()])
            phase2(0, extra=[_iss_wgu(8, 10)])
            phase1(1, extra=[_pair(_iss_wgu(10, 12), _iss_wd(6, 8)),
                             _iss_wd(8, 10), _iss_wd(10, 12)])
            phase2(1)

    nc.compile()
    return nc


def _shard_feed(h16, gp, up, dp, combine, routed, e, piece, C):
    """Build one (expert, FF-half) shard's DMA feeds, pre-laid-out to match the
    kernel's SBUF tile layouts exactly (every DMA contiguous)."""
    r = routed[e]
    n_e = len(r)
    ntb = -(-C // P)
    idx_pad = np.zeros(C, np.int64)
    idx_pad[:n_e] = r
    wt_pad = np.zeros(ntb * P, np.float32)
    wt_pad[:n_e] = combine[e, r]
    hs = slice(piece * FH, (piece + 1) * FH)

    xg = h16[idx_pad]  # [C, D] fp16
    xt_feed = np.ascontiguousarray(xg.reshape(C, DSUB, P).transpose(2, 1, 0))
    wg_feed = gp[e][hs, :].astype(np.float16).reshape(FBH, P, DSUB, P).transpose(0, 3, 2, 1)
    wu_feed = up[e][hs, :].astype(np.float16).reshape(FBH, P, DSUB, P).transpose(0, 3, 2, 1)
    # wgu_feed[fbl, p, g, k, q]: gate/up interleaved so one DMA per fb block
    wgu_feed = np.ascontiguousarray(np.stack([wg_feed, wu_feed], axis=2))
    # wd_feed[fs, p, dt, dn] = down_proj[e][dt*NDN+dn, half*FH + fs*P+p]
    wd_feed = np.ascontiguousarray(
        dp[e][:, hs].astype(np.float16).reshape(NDT, NDN, FBH, P).transpose(2, 3, 0, 1)
    )
    wt_feed = np.ascontiguousarray(wt_pad.reshape(ntb, P).T)
    return xt_feed, wgu_feed, wd_feed, wt_feed


def kernel(hidden_states, top_k_index, top_k_weights, gate_proj, up_proj, down_proj):
    h = np.ascontiguousarray(np.asarray(hidden_states, dtype=np.float32))
    idx = np.asarray(top_k_index)
    wts = np.asarray(top_k_weights, dtype=np.float32)
    gp = np.asarray(gate_proj, dtype=np.float32)
    up = np.asarray(up_proj, dtype=np.float32)
    dp = np.asarray(down_proj, dtype=np.float32)
    assert h.shape == (T, D) and idx.shape == (T, TOPK)
    assert gp.shape == (E, FF, D) and dp.shape == (E, D, FF)

    # combine[e, t] = sum_k wts[t, k] * (idx[t, k] == e)
    combine = np.zeros((E, T), np.float32)
    for k in range(TOPK):
        np.add.at(combine, (idx[:, k], np.arange(T)), wts[:, k])

    routed = [np.nonzero(combine[e] > 0)[0] for e in range(E)]
    cnt = [len(r) for r in routed]

    # E*NSPLIT (expert, FF-piece) shards, sorted by routed count; slot s takes
    # ranks [s*E, (s+1)*E) so every core pairs one shard from each size tier.
    # Ascending: slot 0 = small tier, so the big tier (whose capacity usually
    # has a partial 128-block) is processed last and the final output tile on
    # the critical tail is the small partial one.
    shards = sorted(
        ((e, piece) for e in range(E) for piece in range(NSPLIT)),
        key=lambda sh: cnt[sh[0]],
    )
    slots = [shards[s * E : (s + 1) * E] for s in range(NSPLIT)]
    pad = lambda n: max(P, -(-n // 32) * 32)
    Cs = tuple(int(pad(max(cnt[e] for e, _ in slot))) for slot in slots)

    h16 = h.astype(np.float16)
    in_maps = []
    for core in range(E):
        m = {}
        wgu_parts, wd_parts = [], []
        for s in range(NSPLIT):
            e, piece = slots[s][core]
            xt_f, wgu_f, wd_f, wt_f = _shard_feed(
                h16, gp, up, dp, combine, routed, e, piece, Cs[s]
            )
            m[f"xt{s}"] = xt_f
            m[f"wt{s}"] = wt_f
            wgu_parts.append(wgu_f)
            wd_parts.append(wd_f)
        # [FBLK, P, ...] -> partition-major [P, FBLK, ...] so any [:, b0:b1]
        # block range is a clean row-strided DMA
        m["wgu"] = np.ascontiguousarray(
            np.concatenate(wgu_parts, axis=0).transpose(1, 0, 2, 3, 4)
        )
        m["wd"] = np.ascontiguousarray(
            np.concatenate(wd_parts, axis=0).transpose(1, 0, 2, 3)
        )
        in_maps.append(m)

    ys = _run_on_device(Cs, in_maps)

    out = np.zeros((T, D), np.float32)
    for core in range(E):
        for s in range(NSPLIT):
            e, piece = slots[s][core]
            r = routed[e]
            out[r] += ys[core][s].astype(np.float32)[: len(r)]
    return out


def _have_axon() -> bool:
    """The bass kernel executes via PJRT on the axon-tunneled NeuronCores.
    If the calling process pinned JAX_PLATFORMS=cpu (hiding them), fall back
    to a clean subprocess."""
    try:
        import jax

        return sum(1 for d in jax.devices() if getattr(d, "platform", "") != "cpu") >= E
    except Exception:
        return False


def _run_on_device(Cs: tuple, in_maps: list) -> list:
    global last_results
    if _have_axon():
        if Cs not in _program_cache:
            _program_cache[Cs] = _build_program(Cs)
        nc = _program_cache[Cs]
        last_results = run_bass_kernel_spmd(nc, in_maps, core_ids=list(range(E)))
        return [
            [last_results.results[c][f"y{s}"] for s in range(NSPLIT)]
            for c in range(E)
        ]

    import pickle
    import subprocess
    import tempfile

    d = tempfile.mkdtemp()
    inp, outp = os.path.join(d, "in.pkl"), os.path.join(d, "out.pkl")
    with open(inp, "wb") as f:
        pickle.dump((Cs, in_maps), f)
    env = dict(os.environ)
    env.pop("JAX_PLATFORMS", None)
    subprocess.run(
        [sys.executable, os.path.abspath(__file__), "--device-run", inp, outp],
        check=True,
        env=env,
    )
    with open(outp, "rb") as f:
        return pickle.load(f)


if __name__ == "__main__" and "--device-run" in sys.argv:
    import pickle

    _inp, _outp = sys.argv[2], sys.argv[3]
    with open(_inp, "rb") as f:
        _Cs, _in_maps = pickle.load(f)
    _nc = _build_program(_Cs)
    _res = run_bass_kernel_spmd(_nc, _in_maps, core_ids=list(range(E)))
    with open(_outp, "wb") as f:
        pickle.dump(
            [[_res.results[c][f"y{s}"] for s in range(NSPLIT)] for c in range(E)],
            f,
        )


# revision 31
# speedup vs baseline: 1.0329x; 1.0329x over previous
"""MoE experts kernel for Trainium2 (Bass/Tile), expert-parallel across 8 NeuronCores.

Problem: nn_CompressedMoeExperts — T=2048 tokens, D=1024, FF=1536, E=8 experts,
top-k=2.  out[t] = sum_e combine[e,t] * (silu(h[t] @ Wg[e].T) * (h[t] @ Wu[e].T)) @ Wd[e].T

Sharding: expert-parallel with FF-split load balancing.  Each expert's MLP is
split into two independent shards along the FF dimension (rows of Wg/Wu,
columns of Wd — their partial down-projection outputs simply add).  The 16
shards are sorted by routed-token count and dealt out so every core gets one
"big" and one "small" shard.  Token dispatch (gather by top_k_index) and the
weighted combine scatter-add happen on the host as part of sharding/
unsharding; the combine weight itself is applied on-device.

Matmul operands are fp16 (halves HBM traffic vs fp32, 1 cycle/row on the PE,
fast weight loads), accumulating in fp32 PSUM.  Values are far inside fp16
range and the 10-bit mantissa keeps L2 relative error ~5e-4.

v2 optimizations over the 91.3us baseline (trace-driven):
- Token capacities padded to 32 (not 128): Cs=(544,512) instead of (640,512)
  for the max routed count of 528 → 9K fewer PE cycles (~3.8us).
- Each dma_start costs ~620ns of DIRECT2D issue time serialized on its
  issuing engine's sequencer (72 issues = 44us on Sync in the baseline, and
  ~8us of issue latency before the first weight byte moved).  Fixes: gate+up
  merged into one wgu feed (12 issues not 24), y outputs merged across the
  two D-halves and written fp16 (9-10 issues not 18-20), and issue load split
  across the two HWDGE engines: sync carries wgu/xt/wt, scalar carries wd + y.
- Phase-2 eviction moved from Vector (tensor_scalar_mul, 751ns/tile PSUM read)
  to the Scalar engine as activation(Copy, scale=wt_column) straight to fp16;
  the y DMA is issued by scalar immediately after (same-queue, no cross-engine
  semaphore), halving output bytes.
- Warmup trimmed to 8 matmuls (3.4us at the HAM-gated 1.2GHz exactly covers
  the clock ramp); xt1/wt1 issue hoisted into the startup batch so the s1
  token feed can never serialize behind phase-2 y traffic.
"""

import os
import sys

sys.path.insert(0, "/opt/trn_rl_repo")

import numpy as np

import concourse.bass as bass
import concourse.mybir as mybir
import concourse.tile as tile
from concourse import bacc
from concourse.bass_utils import run_bass_kernel_spmd

# Fixed problem shape
T, D, FF, E, TOPK = 2048, 1024, 1536, 8, 2
P = 128
DSUB = D // P     # 8   k-subtiles over the D contraction
FBLK = FF // P    # 12  128-row blocks over the full FF dimension
NSPLIT = 2
FBH = FBLK // NSPLIT   # 128-row FF blocks per shard
FH = FF // NSPLIT      # FF columns per shard
NDN = 512         # free-dim tile for the down projection
NDT = D // NDN    # 2
NWARM = int(os.environ.get("NWARM", "8"))  # HAM warmup matmuls

F32 = mybir.dt.float32
F16 = mybir.dt.float16

_program_cache: dict[tuple, "bass.Bass"] = {}
last_results = None  # BassKernelResults of the most recent run (for profiling)


def _chunks(C: int) -> list[int]:
    """Split C (multiple of 32) into matmul moving-dim chunks of <=512
    (PSUM bank limit for fp32 accumulation), sizes multiples of 32."""
    n = -(-C // 512)
    base = C // n
    base -= base % 32
    out = [base] * n
    rem = C - base * n  # multiple of 32
    i = 0
    while rem > 0:
        add = min(32, rem)
        out[i % n] += add
        rem -= add
        i += 1
    return sorted(out)  # smallest first: quickest start on freshly-DMAed data


def _build_program(Cs: tuple) -> "bass.Bass":
    nc = bacc.Bacc(None, target_bir_lowering=False)

    ntbs = [-(-C // P) for C in Cs]

    xt_d = [
        nc.dram_tensor(f"xt{s}", [P, DSUB, Cs[s]], F16, kind="ExternalInput")
        for s in range(NSPLIT)
    ]
    wgu_d = nc.dram_tensor("wgu", [P, FBLK, 2, DSUB, P], F16, kind="ExternalInput")
    wd_d = nc.dram_tensor("wd", [P, FBLK, NDT, NDN], F16, kind="ExternalInput")
    wt_d = [
        nc.dram_tensor(f"wt{s}", [P, ntbs[s]], F32, kind="ExternalInput")
        for s in range(NSPLIT)
    ]
    y_d = [
        nc.dram_tensor(f"y{s}", [Cs[s], D], F16, kind="ExternalOutput")
        for s in range(NSPLIT)
    ]

    with tile.TileContext(nc) as tc:
        with (
            tc.tile_pool(name="const", bufs=1) as const_pool,
            tc.tile_pool(name="actp", bufs=1) as act_pool,
            tc.tile_pool(name="sgp", bufs=3) as sg_pool,
            tc.tile_pool(name="yp", bufs=3) as y_pool,
            tc.tile_pool(name="psum", bufs=2, space="PSUM") as psum_pool,
            tc.tile_pool(name="psum_y", bufs=3, space="PSUM") as psum_y_pool,
            tc.tile_pool(name="psum_w", bufs=1, space="PSUM") as psum_w_pool,
        ):
            # HAM pre-warm: dummy matmuls (only dep: the memset) cover the
            # 1.2GHz->2.4GHz clock ramp (~3.4us of PE activity) while the
            # first DMAs stage.
            warm_in = const_pool.tile([P, NDN], F16)
            nc.vector.memset(warm_in[:], 0.0)
            warm_ps = psum_w_pool.tile([P, NDN], F32)
            for _ in range(NWARM):
                nc.tensor.matmul(warm_ps[:], warm_in[:, :P], warm_in[:])

            # Startup DMA batch.  sync (HWDGE) carries wgu/xt/wt; scalar
            # (also HWDGE) carries half of xt0, wd, and later the y outputs.
            # Each dma_start costs ~620ns of DIRECT2D issue time serialized on
            # its engine, while the transfer itself is chopped into
            # per-partition-row descriptors fanned across all 16 hw queues —
            # so FEW, BIG dma_starts both issue fast and use full bandwidth.
            # The whole gate/up table lives in SBUF (6 MB) and streams in as
            # 2-block (1 MB) pieces paced just ahead of the PE.
            # Fine-grained startup: the first real matmul group waits only on
            # wgu block 0 (512K) + xt0 k=0:2 (370K); later k-slices and blocks
            # stream in behind, half on the scalar engine's parallel queue.
            wgu_sb = const_pool.tile([P, FBLK, 2, DSUB, P], F16, name="wgu_sb")
            nc.sync.dma_start(wgu_sb[:, 0:1], wgu_d[:, 0:1])
            xt = [
                const_pool.tile([P, DSUB, Cs[s]], F16, name=f"xt{s}")
                for s in range(NSPLIT)
            ]
            nc.sync.dma_start(xt[0][:, 0:2], xt_d[0][:, 0:2])
            nc.scalar.dma_start(xt[0][:, 4:6], xt_d[0][:, 4:6])
            nc.sync.dma_start(xt[0][:, 2:4], xt_d[0][:, 2:4])
            nc.scalar.dma_start(xt[0][:, 6:8], xt_d[0][:, 6:8])
            wt_sb = [
                const_pool.tile([P, ntbs[s]], F32, name=f"wt{s}") for s in range(NSPLIT)
            ]
            nc.sync.dma_start(wgu_sb[:, 1:2], wgu_d[:, 1:2])
            nc.sync.dma_start(wgu_sb[:, 2:3], wgu_d[:, 2:3])
            nc.sync.dma_start(wgu_sb[:, 3:4], wgu_d[:, 3:4])
            nc.sync.dma_start(wt_sb[0][:], wt_d[0][:])

            wd_sb = const_pool.tile([P, FBLK, NDT, NDN], F16)

            act = [
                act_pool.tile([P, FBH, Cs[s]], F16, name=f"act{s}")
                for s in range(NSPLIT)
            ]

            def phase1(s, extra=()):
                # `extra`: background DMA issues, one per fb block, threaded
                # through the sync queue behind this slot's weight stream.
                C = Cs[s]
                csizes = _chunks(C)
                extra = list(extra)
                for fbl in range(FBH):
                    fb = s * FBH + fbl
                    if fbl < len(extra) and extra[fbl] is not None:
                        extra[fbl]()

                    # slot 0 consumes k-slices in DMA-arrival order: the
                    # scalar-issued pieces (k=4:8) land before sync's (k=0:4),
                    # so the first real matmul starts the moment warmup ends.
                    k_order = (4, 5, 6, 7, 0, 1, 2, 3) if s == 0 else range(DSUB)
                    col = 0
                    for cs in csizes:
                        pg = psum_pool.tile([P, NDN], F32, tag="pg", name="pg")[:, :cs]
                        pu = psum_pool.tile([P, NDN], F32, tag="pu", name="pu")[:, :cs]
                        for i, k in enumerate(k_order):
                            nc.tensor.matmul(
                                pg,
                                wgu_sb[:, fb, 0, k, :],
                                xt[s][:, k, col : col + cs],
                                start=(i == 0),
                                stop=(i == DSUB - 1),
                            )
                        for i, k in enumerate(k_order):
                            nc.tensor.matmul(
                                pu,
                                wgu_sb[:, fb, 1, k, :],
                                xt[s][:, k, col : col + cs],
                                start=(i == 0),
                                stop=(i == DSUB - 1),
                            )
                        sg = sg_pool.tile([P, NDN], F32, tag="sg", name="sg")[:, :cs]
                        nc.scalar.activation(
                            sg, pg, mybir.ActivationFunctionType.Silu
                        )
                        nc.vector.tensor_mul(act[s][:, fbl, col : col + cs], sg, pu)
                        col += cs

            def phase2(s, extra=()):
                # y[t, d] = (actT.T @ WdT_half) * combine_weight[t], evicted on
                # the scalar engine (Copy with per-partition scale) straight to
                # fp16; y DMA issued from scalar right after (same queue).
                # `extra`: background input DMAs to issue one-per-tb — phase2
                # consumes no input stream, so it is the bandwidth-quiet
                # window to stage the next slot's feeds.
                C = Cs[s]
                extra = list(extra)
                per_tb = -(-len(extra) // max(1, ntbs[s]))
                for tb in range(ntbs[s]):
                    for _ in range(per_tb):
                        if extra:
                            extra.pop(0)()
                    rows = min(P, C - tb * P)
                    y_sb = y_pool.tile([P, D], F16, tag="ysb")
                    for dti in range(NDT):
                        py = psum_y_pool.tile([P, NDN], F32, tag="py")
                        for fs in range(FBH):
                            nc.tensor.matmul(
                                py[:rows],
                                act[s][:, fs, tb * P : tb * P + rows],
                                wd_sb[:, s * FBH + fs, dti, :],
                                start=(fs == 0),
                                stop=(fs == FBH - 1),
                            )
                        nc.scalar.activation(
                            y_sb[:rows, dti * NDN : (dti + 1) * NDN],
                            py[:rows],
                            mybir.ActivationFunctionType.Copy,
                            scale=wt_sb[s][:rows, tb : tb + 1],
                        )
                    # y DMA issued from sync (idle during phase 2) so its
                    # DIRECT2D overlaps the scalar evictions
                    nc.sync.dma_start(
                        y_d[s][tb * P : tb * P + rows, :], y_sb[:rows]
                    )

            def _iss_xt1(j0, j1):
                return lambda: nc.sync.dma_start(
                    xt[1][:, j0:j1], xt_d[1][:, j0:j1]
                )

            def _iss_wt1():
                return lambda: nc.sync.dma_start(wt_sb[1][:], wt_d[1][:])

            def _iss_wgu(b0, b1):
                return lambda: nc.sync.dma_start(
                    wgu_sb[:, b0:b1], wgu_d[:, b0:b1]
                )

            def _iss_wd(b0, b1):
                return lambda: nc.scalar.dma_start(
                    wd_sb[:, b0:b1], wd_d[:, b0:b1]
                )

            # Input-stream pacing (~1 MB per issue, ~2 pieces of lookahead):
            # the startup batch carries wgu 0:4 + xt0; phase1(0) threads in
            # wgu 4:8, the xt1 halves, and (on scalar) wd 0:6 which phase2(0)
            # needs; phase2(0)'s input-quiet window stages wgu 8:10;
            # phase1(1) only needs wgu 10:12 + wd 6:12.
            def _pair(a, b):
                return lambda: (a(), b())

            phase1(0, extra=[_iss_wgu(4, 6), _iss_xt1(0, 4),
                             _pair(_iss_wgu(6, 8), _iss_wd(0, 2)),
                             _pair(_iss_xt1(4, 8), _iss_wd(2, 4)),
                             _iss_wd(4, 6), _iss_wt1()])
            phase2(0, extra=[_iss_wgu(8, 10)])
            phase1(1, extra=[_pair(_iss_wgu(10, 12), _iss_wd(6, 8)),
                             _iss_wd(8, 10), _iss_wd(10, 12)])
            phase2(1)

    nc.compile()
    return nc


def _shard_feed(h16, gp, up, dp, combine, routed, e, piece, C):
    """Build one (expert, FF-half) shard's DMA feeds, pre-laid-out to match the
    kernel's SBUF tile layouts exactly (every DMA contiguous)."""
    r = routed[e]
    n_e = len(r)
    ntb = -(-C // P)
    idx_pad = np.zeros(C, np.int64)
    idx_pad[:n_e] = r
    wt_pad = np.zeros(ntb * P, np.float32)
    wt_pad[:n_e] = combine[e, r]
    hs = slice(piece * FH, (piece + 1) * FH)

    xg = h16[idx_pad]  # [C, D] fp16
    xt_feed = np.ascontiguousarray(xg.reshape(C, DSUB, P).transpose(2, 1, 0))
    wg_feed = gp[e][hs, :].astype(np.float16).reshape(FBH, P, DSUB, P).transpose(0, 3, 2, 1)
    wu_feed = up[e][hs, :].astype(np.float16).reshape(FBH, P, DSUB, P).transpose(0, 3, 2, 1)
    # wgu_feed[fbl, p, g, k, q]: gate/up interleaved so one DMA per fb block
    wgu_feed = np.ascontiguousarray(np.stack([wg_feed, wu_feed], axis=2))
    # wd_feed[fs, p, dt, dn] = down_proj[e][dt*NDN+dn, half*FH + fs*P+p]
    wd_feed = np.ascontiguousarray(
        dp[e][:, hs].astype(np.float16).reshape(NDT, NDN, FBH, P).transpose(2, 3, 0, 1)
    )
    wt_feed = np.ascontiguousarray(wt_pad.reshape(ntb, P).T)
    return xt_feed, wgu_feed, wd_feed, wt_feed


def kernel(hidden_states, top_k_index, top_k_weights, gate_proj, up_proj, down_proj):
    h = np.ascontiguousarray(np.asarray(hidden_states, dtype=np.float32))
    idx = np.asarray(top_k_index)
    wts = np.asarray(top_k_weights, dtype=np.float32)
    gp = np.asarray(gate_proj, dtype=np.float32)
    up = np.asarray(up_proj, dtype=np.float32)
    dp = np.asarray(down_proj, dtype=np.float32)
    assert h.shape == (T, D) and idx.shape == (T, TOPK)
    assert gp.shape == (E, FF, D) and dp.shape == (E, D, FF)

    # combine[e, t] = sum_k wts[t, k] * (idx[t, k] == e)
    combine = np.zeros((E, T), np.float32)
    for k in range(TOPK):
        np.add.at(combine, (idx[:, k], np.arange(T)), wts[:, k])

    routed = [np.nonzero(combine[e] > 0)[0] for e in range(E)]
    cnt = [len(r) for r in routed]

    # E*NSPLIT (expert, FF-piece) shards, sorted by routed count; slot s takes
    # ranks [s*E, (s+1)*E) so every core pairs one shard from each size tier.
    # Ascending: slot 0 = small tier, so the big tier (whose capacity usually
    # has a partial 128-block) is processed last and the final output tile on
    # the critical tail is the small partial one.
    shards = sorted(
        ((e, piece) for e in range(E) for piece in range(NSPLIT)),
        key=lambda sh: cnt[sh[0]],
    )
    slots = [shards[s * E : (s + 1) * E] for s in range(NSPLIT)]
    pad = lambda n: max(P, -(-n // 32) * 32)
    Cs = tuple(int(pad(max(cnt[e] for e, _ in slot))) for slot in slots)

    h16 = h.astype(np.float16)
    in_maps = []
    for core in range(E):
        m = {}
        wgu_parts, wd_parts = [], []
        for s in range(NSPLIT):
            e, piece = slots[s][core]
            xt_f, wgu_f, wd_f, wt_f = _shard_feed(
                h16, gp, up, dp, combine, routed, e, piece, Cs[s]
            )
            m[f"xt{s}"] = xt_f
            m[f"wt{s}"] = wt_f
            wgu_parts.append(wgu_f)
            wd_parts.append(wd_f)
        # [FBLK, P, ...] -> partition-major [P, FBLK, ...] so any [:, b0:b1]
        # block range is a clean row-strided DMA
        m["wgu"] = np.ascontiguousarray(
            np.concatenate(wgu_parts, axis=0).transpose(1, 0, 2, 3, 4)
        )
        m["wd"] = np.ascontiguousarray(
            np.concatenate(wd_parts, axis=0).transpose(1, 0, 2, 3)
        )
        in_maps.append(m)

    ys = _run_on_device(Cs, in_maps)

    out = np.zeros((T, D), np.float32)
    for core in range(E):
        for s in range(NSPLIT):
            e, piece = slots[s][core]
            r = routed[e]
            out[r] += ys[core][s].astype(np.float32)[: len(r)]
    return out


def _have_axon() -> bool:
    """The bass kernel executes via PJRT on the axon-tunneled NeuronCores.
    If the calling process pinned JAX_PLATFORMS=cpu (hiding them), fall back
    to a clean subprocess."""
    try:
        import jax

        return sum(1 for d in jax.devices() if getattr(d, "platform", "") != "cpu") >= E
    except Exception:
        return False


def _run_on_device(Cs: tuple, in_maps: list) -> list:
    global last_results
    if _have_axon():
        if Cs not in _program_cache:
            _program_cache[Cs] = _build_program(Cs)
        nc = _program_cache[Cs]
        last_results = run_bass_kernel_spmd(nc, in_maps, core_ids=list(range(E)))
        return [
            [last_results.results[c][f"y{s}"] for s in range(NSPLIT)]
            for c in range(E)
        ]

    import pickle
    import subprocess
    import tempfile

    d = tempfile.mkdtemp()
    inp, outp = os.path.join(d, "in.pkl"), os.path.join(d, "out.pkl")
    with open(inp, "wb") as f:
        pickle.dump((Cs, in_maps), f)
    env = dict(os.environ)
    env.pop("JAX_PLATFORMS", None)
    subprocess.run(
        [sys.executable, os.path.abspath(__file__), "--device-run", inp, outp],
        check=True,
        env=env,
    )
    with open(outp, "rb") as f:
        return pickle.load(f)


if __name__ == "__main__" and "--device-run" in sys.argv:
    import pickle

    _inp, _outp = sys.argv[2], sys.argv[3]
    with open(_inp, "rb") as f:
        _Cs, _in_maps = pickle.load(f)
    _nc = _build_program(_Cs)
    _res = run_bass_kernel_spmd(_nc, _in_maps, core_ids=list(range(E)))
    with open(_outp, "wb") as f:
        pickle.dump(
            [[_res.results[c][f"y{s}"] for s in range(NSPLIT)] for c in range(E)],
            f,
        )


# revision 32
# speedup vs baseline: 1.0524x; 1.0188x over previous
"""MoE experts kernel for Trainium2 (Bass/Tile), expert-parallel across 8 NeuronCores.

Problem: nn_CompressedMoeExperts — T=2048 tokens, D=1024, FF=1536, E=8 experts,
top-k=2.  out[t] = sum_e combine[e,t] * (silu(h[t] @ Wg[e].T) * (h[t] @ Wu[e].T)) @ Wd[e].T

Sharding: expert-parallel with FF-split load balancing.  Each expert's MLP is
split into two independent shards along the FF dimension (rows of Wg/Wu,
columns of Wd — their partial down-projection outputs simply add).  The 16
shards are sorted by routed-token count and dealt out so every core gets one
"big" and one "small" shard.  Token dispatch (gather by top_k_index) and the
weighted combine scatter-add happen on the host as part of sharding/
unsharding; the combine weight itself is applied on-device.

Matmul operands are fp16 (halves HBM traffic vs fp32, 1 cycle/row on the PE,
fast weight loads), accumulating in fp32 PSUM.  Values are far inside fp16
range and the 10-bit mantissa keeps L2 relative error ~5e-4.

v2 optimizations over the 91.3us baseline (trace-driven):
- Token capacities padded to 32 (not 128): Cs=(544,512) instead of (640,512)
  for the max routed count of 528 → 9K fewer PE cycles (~3.8us).
- Each dma_start costs ~620ns of DIRECT2D issue time serialized on its
  issuing engine's sequencer (72 issues = 44us on Sync in the baseline, and
  ~8us of issue latency before the first weight byte moved).  Fixes: gate+up
  merged into one wgu feed (12 issues not 24), y outputs merged across the
  two D-halves and written fp16 (9-10 issues not 18-20), and issue load split
  across the two HWDGE engines: sync carries wgu/xt/wt, scalar carries wd + y.
- Phase-2 eviction moved from Vector (tensor_scalar_mul, 751ns/tile PSUM read)
  to the Scalar engine as activation(Copy, scale=wt_column) straight to fp16;
  the y DMA is issued by scalar immediately after (same-queue, no cross-engine
  semaphore), halving output bytes.
- Warmup trimmed to 8 matmuls (3.4us at the HAM-gated 1.2GHz exactly covers
  the clock ramp); xt1/wt1 issue hoisted into the startup batch so the s1
  token feed can never serialize behind phase-2 y traffic.
"""

import os
import sys

sys.path.insert(0, "/opt/trn_rl_repo")

import numpy as np

import concourse.bass as bass
import concourse.mybir as mybir
import concourse.tile as tile
from concourse import bacc
from concourse.bass_utils import run_bass_kernel_spmd

# Fixed problem shape
T, D, FF, E, TOPK = 2048, 1024, 1536, 8, 2
P = 128
DSUB = D // P     # 8   k-subtiles over the D contraction
FBLK = FF // P    # 12  128-row blocks over the full FF dimension
NSPLIT = 2
FBH = FBLK // NSPLIT   # 128-row FF blocks per shard
FH = FF // NSPLIT      # FF columns per shard
NDN = 512         # free-dim tile for the down projection
NDT = D // NDN    # 2
NWARM = int(os.environ.get("NWARM", "8"))  # HAM warmup matmuls

F32 = mybir.dt.float32
F16 = mybir.dt.float16

_program_cache: dict[tuple, "bass.Bass"] = {}
last_results = None  # BassKernelResults of the most recent run (for profiling)


def _chunks(C: int) -> list[int]:
    """Split C (multiple of 32) into matmul moving-dim chunks of <=512
    (PSUM bank limit for fp32 accumulation), sizes multiples of 32."""
    n = -(-C // 512)
    base = C // n
    base -= base % 32
    out = [base] * n
    rem = C - base * n  # multiple of 32
    i = 0
    while rem > 0:
        add = min(32, rem)
        out[i % n] += add
        rem -= add
        i += 1
    return sorted(out)  # smallest first: quickest start on freshly-DMAed data


def _build_program(Cs: tuple) -> "bass.Bass":
    nc = bacc.Bacc(None, target_bir_lowering=False)

    ntbs = [-(-C // P) for C in Cs]

    xt_d = [
        nc.dram_tensor(f"xt{s}", [P, DSUB, Cs[s]], F16, kind="ExternalInput")
        for s in range(NSPLIT)
    ]
    wgu_d = nc.dram_tensor("wgu", [P, FBLK, 2, DSUB, P], F16, kind="ExternalInput")
    wd_d = nc.dram_tensor("wd", [P, FBLK, NDT, NDN], F16, kind="ExternalInput")
    wt_d = [
        nc.dram_tensor(f"wt{s}", [P, ntbs[s]], F32, kind="ExternalInput")
        for s in range(NSPLIT)
    ]
    y_d = [
        nc.dram_tensor(f"y{s}", [Cs[s], D], F16, kind="ExternalOutput")
        for s in range(NSPLIT)
    ]

    with tile.TileContext(nc) as tc:
        with (
            tc.tile_pool(name="const", bufs=1) as const_pool,
            tc.tile_pool(name="actp", bufs=1) as act_pool,
            tc.tile_pool(name="sgp", bufs=3) as sg_pool,
            tc.tile_pool(name="yp", bufs=3) as y_pool,
            tc.tile_pool(name="psum", bufs=2, space="PSUM") as psum_pool,
            tc.tile_pool(name="psum_y", bufs=3, space="PSUM") as psum_y_pool,
            tc.tile_pool(name="psum_w", bufs=1, space="PSUM") as psum_w_pool,
        ):
            # HAM pre-warm: dummy matmuls (only dep: the memset) cover the
            # 1.2GHz->2.4GHz clock ramp (~3.4us of PE activity) while the
            # first DMAs stage.
            warm_in = const_pool.tile([P, NDN], F16)
            nc.vector.memset(warm_in[:], 0.0)
            warm_ps = psum_w_pool.tile([P, NDN], F32)
            for _ in range(NWARM):
                nc.tensor.matmul(warm_ps[:], warm_in[:, :P], warm_in[:])

            # Startup DMA batch.  sync (HWDGE) carries wgu/xt/wt; scalar
            # (also HWDGE) carries half of xt0, wd, and later the y outputs.
            # Each dma_start costs ~620ns of DIRECT2D issue time serialized on
            # its engine, while the transfer itself is chopped into
            # per-partition-row descriptors fanned across all 16 hw queues —
            # so FEW, BIG dma_starts both issue fast and use full bandwidth.
            # The whole gate/up table lives in SBUF (6 MB) and streams in as
            # 2-block (1 MB) pieces paced just ahead of the PE.
            # Fine-grained startup: the first real matmul group waits only on
            # wgu block 0 (512K) + xt0 k=0:2 (370K); later k-slices and blocks
            # stream in behind, half on the scalar engine's parallel queue.
            wgu_sb = const_pool.tile([P, FBLK, 2, DSUB, P], F16, name="wgu_sb")
            nc.sync.dma_start(wgu_sb[:, 0:1], wgu_d[:, 0:1])
            xt = [
                const_pool.tile([P, DSUB, Cs[s]], F16, name=f"xt{s}")
                for s in range(NSPLIT)
            ]
            nc.sync.dma_start(xt[0][:, 0:2], xt_d[0][:, 0:2])
            nc.scalar.dma_start(xt[0][:, 4:6], xt_d[0][:, 4:6])
            nc.sync.dma_start(xt[0][:, 2:4], xt_d[0][:, 2:4])
            nc.scalar.dma_start(xt[0][:, 6:8], xt_d[0][:, 6:8])
            wt_sb = [
                const_pool.tile([P, ntbs[s]], F32, name=f"wt{s}") for s in range(NSPLIT)
            ]
            nc.sync.dma_start(wgu_sb[:, 1:2], wgu_d[:, 1:2])
            nc.sync.dma_start(wgu_sb[:, 2:3], wgu_d[:, 2:3])
            nc.sync.dma_start(wgu_sb[:, 3:4], wgu_d[:, 3:4])
            nc.sync.dma_start(wt_sb[0][:], wt_d[0][:])

            wd_sb = const_pool.tile([P, FBLK, NDT, NDN], F16)

            act = [
                act_pool.tile([P, FBH, Cs[s]], F16, name=f"act{s}")
                for s in range(NSPLIT)
            ]

            def phase1(s, extra=()):
                # `extra`: background DMA issues, one per fb block, threaded
                # through the sync queue behind this slot's weight stream.
                C = Cs[s]
                csizes = _chunks(C)
                extra = list(extra)
                for fbl in range(FBH):
                    fb = s * FBH + fbl
                    if fbl < len(extra) and extra[fbl] is not None:
                        extra[fbl]()

                    # slot 0 consumes k-slices in DMA-arrival order: the
                    # scalar-issued pieces (k=4:8) land before sync's (k=0:4),
                    # so the first real matmul starts the moment warmup ends.
                    k_order = (4, 5, 6, 7, 0, 1, 2, 3) if s == 0 else range(DSUB)
                    col = 0
                    for cs in csizes:
                        pg = psum_pool.tile([P, NDN], F32, tag="pg", name="pg")[:, :cs]
                        pu = psum_pool.tile([P, NDN], F32, tag="pu", name="pu")[:, :cs]
                        for i, k in enumerate(k_order):
                            nc.tensor.matmul(
                                pg,
                                wgu_sb[:, fb, 0, k, :],
                                xt[s][:, k, col : col + cs],
                                start=(i == 0),
                                stop=(i == DSUB - 1),
                            )
                        for i, k in enumerate(k_order):
                            nc.tensor.matmul(
                                pu,
                                wgu_sb[:, fb, 1, k, :],
                                xt[s][:, k, col : col + cs],
                                start=(i == 0),
                                stop=(i == DSUB - 1),
                            )
                        sg = sg_pool.tile([P, NDN], F32, tag="sg", name="sg")[:, :cs]
                        nc.scalar.activation(
                            sg, pg, mybir.ActivationFunctionType.Silu
                        )
                        nc.vector.tensor_mul(act[s][:, fbl, col : col + cs], sg, pu)
                        col += cs

            def phase2(s, extra=()):
                # y[t, d] = (actT.T @ WdT_half) * combine_weight[t], evicted on
                # the scalar engine (Copy with per-partition scale) straight to
                # fp16; y DMA issued from scalar right after (same queue).
                # `extra`: background input DMAs to issue one-per-tb — phase2
                # consumes no input stream, so it is the bandwidth-quiet
                # window to stage the next slot's feeds.
                C = Cs[s]
                extra = list(extra)
                per_tb = -(-len(extra) // max(1, ntbs[s]))
                for tb in range(ntbs[s]):
                    for _ in range(per_tb):
                        if extra:
                            extra.pop(0)()
                    rows = min(P, C - tb * P)
                    last = s == NSPLIT - 1 and tb == ntbs[s] - 1
                    y_sb = y_pool.tile([P, D], F16, tag="ysb")
                    for dti in range(NDT):
                        py = psum_y_pool.tile([P, NDN], F32, tag="py")
                        for fs in range(FBH):
                            nc.tensor.matmul(
                                py[:rows],
                                act[s][:, fs, tb * P : tb * P + rows],
                                wd_sb[:, s * FBH + fs, dti, :],
                                start=(fs == 0),
                                stop=(fs == FBH - 1),
                            )
                        nc.scalar.activation(
                            y_sb[:rows, dti * NDN : (dti + 1) * NDN],
                            py[:rows],
                            mybir.ActivationFunctionType.Copy,
                            scale=wt_sb[s][:rows, tb : tb + 1],
                        )
                        if last:
                            # final tile: DMA each D-half as soon as its
                            # eviction lands, so only the second (tiny) half
                            # sits on the critical tail
                            nc.sync.dma_start(
                                y_d[s][
                                    tb * P : tb * P + rows,
                                    dti * NDN : (dti + 1) * NDN,
                                ],
                                y_sb[:rows, dti * NDN : (dti + 1) * NDN],
                            )
                    # y DMA issued from sync (idle during phase 2) so its
                    # DIRECT2D overlaps the scalar evictions
                    if not last:
                        nc.sync.dma_start(
                            y_d[s][tb * P : tb * P + rows, :], y_sb[:rows]
                        )

            def _iss_xt1(j0, j1):
                return lambda: nc.sync.dma_start(
                    xt[1][:, j0:j1], xt_d[1][:, j0:j1]
                )

            def _iss_wt1():
                return lambda: nc.sync.dma_start(wt_sb[1][:], wt_d[1][:])

            def _iss_wgu(b0, b1):
                return lambda: nc.sync.dma_start(
                    wgu_sb[:, b0:b1], wgu_d[:, b0:b1]
                )

            def _iss_wd(b0, b1):
                return lambda: nc.scalar.dma_start(
                    wd_sb[:, b0:b1], wd_d[:, b0:b1]
                )

            # Input-stream pacing (~1 MB per issue, ~2 pieces of lookahead):
            # the startup batch carries wgu 0:4 + xt0; phase1(0) threads in
            # wgu 4:8, the xt1 halves, and (on scalar) wd 0:6 which phase2(0)
            # needs; phase2(0)'s input-quiet window stages wgu 8:10;
            # phase1(1) only needs wgu 10:12 + wd 6:12.
            def _pair(a, b):
                return lambda: (a(), b())

            phase1(0, extra=[_iss_wgu(4, 6), _iss_xt1(0, 4),
                             _pair(_iss_wgu(6, 8), _iss_wd(0, 2)),
                             _pair(_iss_xt1(4, 8), _iss_wd(2, 4)),
                             _iss_wd(4, 6), _iss_wt1()])
            phase2(0, extra=[_iss_wgu(8, 10)])
            phase1(1, extra=[_pair(_iss_wgu(10, 12), _iss_wd(6, 8)),
                             _iss_wd(8, 10), _iss_wd(10, 12)])
            phase2(1)

    nc.compile()
    return nc


def _shard_feed(h16, gp, up, dp, combine, routed, e, piece, C):
    """Build one (expert, FF-half) shard's DMA feeds, pre-laid-out to match the
    kernel's SBUF tile layouts exactly (every DMA contiguous)."""
    r = routed[e]
    n_e = len(r)
    ntb = -(-C // P)
    idx_pad = np.zeros(C, np.int64)
    idx_pad[:n_e] = r
    wt_pad = np.zeros(ntb * P, np.float32)
    wt_pad[:n_e] = combine[e, r]
    hs = slice(piece * FH, (piece + 1) * FH)

    xg = h16[idx_pad]  # [C, D] fp16
    xt_feed = np.ascontiguousarray(xg.reshape(C, DSUB, P).transpose(2, 1, 0))
    wg_feed = gp[e][hs, :].astype(np.float16).reshape(FBH, P, DSUB, P).transpose(0, 3, 2, 1)
    wu_feed = up[e][hs, :].astype(np.float16).reshape(FBH, P, DSUB, P).transpose(0, 3, 2, 1)
    # wgu_feed[fbl, p, g, k, q]: gate/up interleaved so one DMA per fb block
    wgu_feed = np.ascontiguousarray(np.stack([wg_feed, wu_feed], axis=2))
    # wd_feed[fs, p, dt, dn] = down_proj[e][dt*NDN+dn, half*FH + fs*P+p]
    wd_feed = np.ascontiguousarray(
        dp[e][:, hs].astype(np.float16).reshape(NDT, NDN, FBH, P).transpose(2, 3, 0, 1)
    )
    wt_feed = np.ascontiguousarray(wt_pad.reshape(ntb, P).T)
    return xt_feed, wgu_feed, wd_feed, wt_feed


def kernel(hidden_states, top_k_index, top_k_weights, gate_proj, up_proj, down_proj):
    h = np.ascontiguousarray(np.asarray(hidden_states, dtype=np.float32))
    idx = np.asarray(top_k_index)
    wts = np.asarray(top_k_weights, dtype=np.float32)
    gp = np.asarray(gate_proj, dtype=np.float32)
    up = np.asarray(up_proj, dtype=np.float32)
    dp = np.asarray(down_proj, dtype=np.float32)
    assert h.shape == (T, D) and idx.shape == (T, TOPK)
    assert gp.shape == (E, FF, D) and dp.shape == (E, D, FF)

    # combine[e, t] = sum_k wts[t, k] * (idx[t, k] == e)
    combine = np.zeros((E, T), np.float32)
    for k in range(TOPK):
        np.add.at(combine, (idx[:, k], np.arange(T)), wts[:, k])

    routed = [np.nonzero(combine[e] > 0)[0] for e in range(E)]
    cnt = [len(r) for r in routed]

    # E*NSPLIT (expert, FF-piece) shards, sorted by routed count; slot s takes
    # ranks [s*E, (s+1)*E) so every core pairs one shard from each size tier.
    # Ascending: slot 0 = small tier, so the big tier (whose capacity usually
    # has a partial 128-block) is processed last and the final output tile on
    # the critical tail is the small partial one.
    shards = sorted(
        ((e, piece) for e in range(E) for piece in range(NSPLIT)),
        key=lambda sh: cnt[sh[0]],
    )
    slots = [shards[s * E : (s + 1) * E] for s in range(NSPLIT)]
    pad = lambda n: max(P, -(-n // 32) * 32)
    Cs = tuple(int(pad(max(cnt[e] for e, _ in slot))) for slot in slots)

    h16 = h.astype(np.float16)
    in_maps = []
    for core in range(E):
        m = {}
        wgu_parts, wd_parts = [], []
        for s in range(NSPLIT):
            e, piece = slots[s][core]
            xt_f, wgu_f, wd_f, wt_f = _shard_feed(
                h16, gp, up, dp, combine, routed, e, piece, Cs[s]
            )
            m[f"xt{s}"] = xt_f
            m[f"wt{s}"] = wt_f
            wgu_parts.append(wgu_f)
            wd_parts.append(wd_f)
        # [FBLK, P, ...] -> partition-major [P, FBLK, ...] so any [:, b0:b1]
        # block range is a clean row-strided DMA
        m["wgu"] = np.ascontiguousarray(
            np.concatenate(wgu_parts, axis=0).transpose(1, 0, 2, 3, 4)
        )
        m["wd"] = np.ascontiguousarray(
            np.concatenate(wd_parts, axis=0).transpose(1, 0, 2, 3)
        )
        in_maps.append(m)

    ys = _run_on_device(Cs, in_maps)

    out = np.zeros((T, D), np.float32)
    for core in range(E):
        for s in range(NSPLIT):
            e, piece = slots[s][core]
            r = routed[e]
            out[r] += ys[core][s].astype(np.float32)[: len(r)]
    return out


def _have_axon() -> bool:
    """The bass kernel executes via PJRT on the axon-tunneled NeuronCores.
    If the calling process pinned JAX_PLATFORMS=cpu (hiding them), fall back
    to a clean subprocess."""
    try:
        import jax

        return sum(1 for d in jax.devices() if getattr(d, "platform", "") != "cpu") >= E
    except Exception:
        return False


def _run_on_device(Cs: tuple, in_maps: list) -> list:
    global last_results
    if _have_axon():
        if Cs not in _program_cache:
            _program_cache[Cs] = _build_program(Cs)
        nc = _program_cache[Cs]
        last_results = run_bass_kernel_spmd(nc, in_maps, core_ids=list(range(E)))
        return [
            [last_results.results[c][f"y{s}"] for s in range(NSPLIT)]
            for c in range(E)
        ]

    import pickle
    import subprocess
    import tempfile

    d = tempfile.mkdtemp()
    inp, outp = os.path.join(d, "in.pkl"), os.path.join(d, "out.pkl")
    with open(inp, "wb") as f:
        pickle.dump((Cs, in_maps), f)
    env = dict(os.environ)
    env.pop("JAX_PLATFORMS", None)
    subprocess.run(
        [sys.executable, os.path.abspath(__file__), "--device-run", inp, outp],
        check=True,
        env=env,
    )
    with open(outp, "rb") as f:
        return pickle.load(f)


if __name__ == "__main__" and "--device-run" in sys.argv:
    import pickle

    _inp, _outp = sys.argv[2], sys.argv[3]
    with open(_inp, "rb") as f:
        _Cs, _in_maps = pickle.load(f)
    _nc = _build_program(_Cs)
    _res = run_bass_kernel_spmd(_nc, _in_maps, core_ids=list(range(E)))
    with open(_outp, "wb") as f:
        pickle.dump(
            [[_res.results[c][f"y{s}"] for s in range(NSPLIT)] for c in range(E)],
            f,
        )
